# revision 8
# baseline (speedup 1.0000x reference)
"""Differentiable SVM (hinge-loss GD + linear predict) on 8 Trainium2 cores.

Strategy (v3 — ReduceScatter + AllGather, W-slice update):
  - Support rows sharded 512/core; scores/hinge/local-partial-gradient
    computed per core (gl^T X_c, classes-major, 16 matmuls of free-512).
  - Per GD iteration, per embed-half: ReduceScatter sums the partial
    gradient blobs (each core receives a 16-class slice), the core updates
    its f32 W^T-slice master (stored folded [128,128] so DVE uses all
    lanes), casts to bf16, and AllGathers the updated W^T half. The AG
    output is xbar-transposed (one dma_start_transpose) straight into the
    embed-major k-tile layout the scores/query matmuls consume.
  - All collective blobs are contiguous [128|16, F] rows: every pack and
    unpack is a <=128-fat-descriptor DMA. AllReduce was measured 3.5x less
    efficient per byte than AllGather on this transport, hence RS+AG.
  - The bias gradient rides the half-B blob as a column block (col 1024 of
    a padded 128-col block); the updated bf16 bias row rides the half-B AG
    the same way and lands as block 8 of the xbar output, read as [1,128].
  - Iteration 0 is closed-form (W=0 => G0 = 1-128*oh): W_1 computed
    host-side; the device runs iterations 1..14.
  - Query phase: out^T = W^T Q^T + b, k-major over 4 chunks of 512; Q^T
    slices prefetched one per iteration during the fit.
"""
import os

import numpy as np
import ml_dtypes

import concourse.bass as bass
import concourse.bacc as bacc
import concourse.masks as masks
import concourse.mybir as mybir
import concourse.tile as tile
from concourse.bass_utils import run_bass_kernel_spmd

BF16 = ml_dtypes.bfloat16
F32 = mybir.dt.float32
BF = mybir.dt.bfloat16
ALU = mybir.AluOpType

NCORES = 8
N_SUP = 4096
D = 2048
KCLS = 128
N_Q = 16384
SROWS = N_SUP // NCORES     # 512 support rows / core (4 row k-tiles)
QROWS = N_Q // NCORES       # 2048 query rows / core (4 chunks of 512)
ITERS = 15                  # total GD iterations; it 0 is host-side
LR = np.float32(0.01)
CREG = np.float32(1.0)
NK = np.float32(N_SUP * KCLS)
DECAY = float(np.float32(1.0) - LR * CREG)   # 0.99
LRNK = float(LR / NK)
KT = D // 128               # 16 embed k-tiles
RT = SROWS // 128           # 4 support-row k-tiles
HW_ = D // 2                # 1024 embed cols per half
BW = HW_ + KCLS             # half-B blob width (bias col block appended)
CS = KCLS // NCORES         # 16-class slice per core
GROUP = [list(range(NCORES))]


def build():
    nc = bacc.Bacc("TRN2", target_bir_lowering=False, debug=False,
                   num_devices=NCORES)

    xst = nc.dram_tensor("xst", [128, KT * SROWS], BF, kind="ExternalInput")
    xloc = nc.dram_tensor("xloc", [128, RT * D], BF, kind="ExternalInput")
    oht = nc.dram_tensor("oht", [128, RT * KCLS], BF, kind="ExternalInput")
    qtt = nc.dram_tensor("qtt", [128, KT * QROWS], BF, kind="ExternalInput")
    w1a = nc.dram_tensor("w1a", [128, HW_], BF, kind="ExternalInput")
    w1b_ = nc.dram_tensor("w1b_", [128, HW_], BF, kind="ExternalInput")
    w1t0 = nc.dram_tensor("w1t0", [128, 128], F32, kind="ExternalInput")
    w1t1 = nc.dram_tensor("w1t1", [128, 128], F32, kind="ExternalInput")
    w1brf = nc.dram_tensor("w1brf", [1, KCLS], F32, kind="ExternalInput")
    w1brb = nc.dram_tensor("w1brb", [1, KCLS], BF, kind="ExternalInput")
    outT = nc.dram_tensor("outT", [KCLS, QROWS], F32, kind="ExternalOutput")

    with tile.TileContext(nc) as tc:
        with (
            tc.tile_pool(name="static", bufs=1) as st,
            tc.tile_pool(name="dram", bufs=1, space="DRAM") as dram,
            tc.tile_pool(name="small", bufs=8) as sm,
            tc.tile_pool(name="scratch", bufs=4) as scr,
        ):
            # ---- static SBUF tensors ----
            xst_sb = st.tile([128, KT * SROWS], BF)     # X_c^T k-tiles
            xloc_sb = st.tile([128, RT * D], BF)        # X_c row k-tiles
            oh_sb = st.tile([128, RT * KCLS], BF)       # one-hot row k-tiles
            qt_sb = st.tile([128, KT * QROWS], BF)      # Q_c^T (prefetched)
            w_sbA = st.tile([128, HW_], BF)             # W k-tiles 0..7
            w_sbB = st.tile([128, HW_], BF)             # W k-tiles 8..15
            wb_f32 = st.tile([1, KCLS], F32)            # bias row master
            wb_bf = st.tile([1, KCLS], BF)              # bias row bf16
            wts0 = st.tile([128, 128], F32)             # W^T slice A folded
            wts1 = st.tile([128, 128], F32)             # W^T slice B folded
            gsA = st.tile([128, 128], BF)               # RS result A folded
            gsB = st.tile([128, 128], BF)               # RS result B folded
            gbk = st.tile([128, 8], BF)                 # bias grad pack
            gbfl = st.tile([1, 1024], BF)               # bias AR result flat
            gbst = st.tile([1, KCLS], BF)               # bias grad row
            wc0 = st.tile([128, 128], BF)               # cast W slice A
            wc1 = st.tile([128, 128], BF)               # cast W slice B
            gl_sb = st.tile([128, RT * KCLS], BF)       # -NK*G local
            sT_sb = st.tile([128, SROWS], BF)           # scores^T bf16
            gpk0 = st.tile([128, HW_], BF)              # RS pack half A
            gpk1 = st.tile([128, HW_], BF)              # RS pack half B
            ones_c = st.tile([128, 1], BF)              # ones col (gradb rhs)
            ones_r = st.tile([1, SROWS], BF)            # ones row (bias rhs)
            id_bf = st.tile([128, 128], BF)

            nc.vector.memset(gbk[:], 0.0)  # cols 1..7 stay zero
            nc.vector.memset(ones_c[:], 1.0)
            nc.vector.memset(ones_r[:], 1.0)
            masks.make_identity(nc, id_bf[:])

            # ---- initial loads (host pre-tiled: fat descriptors only) ----
            nc.sync.dma_start(w_sbA[:], w1a[:])
            nc.sync.dma_start(w_sbB[:], w1b_[:])
            for lo, hi in ((0, 4), (4, 8), (8, 12), (12, 16)):
                nc.sync.dma_start(xst_sb[:, lo * SROWS:hi * SROWS],
                                  xst[:, lo * SROWS:hi * SROWS])
            nc.sync.dma_start(oh_sb[:], oht[:])
            for lo, hi in ((0, 2), (2, 4)):
                nc.scalar.dma_start(xloc_sb[:, lo * D:hi * D],
                                    xloc[:, lo * D:hi * D])
            nc.scalar.dma_start(wts0[:], w1t0[:])
            nc.scalar.dma_start(wts1[:], w1t1[:])
            nc.scalar.dma_start(wb_f32[:], w1brf[:])
            nc.scalar.dma_start(wb_bf[:], w1brb[:])

            with (
                tc.tile_pool(name="ps_sc", bufs=1, space="PSUM") as ps_sc,
                tc.tile_pool(name="ps_m", bufs=2, space="PSUM") as ps_m,
                tc.tile_pool(name="ps_g", bufs=2, space="PSUM") as ps_g,
                tc.tile_pool(name="ps_b", bufs=1, space="PSUM") as ps_b,
            ):
                for it in range(1, ITERS):
                    # ---- scores^T = W^T X_c^T + b : [classes, 512] ----
                    psT = ps_sc.tile([128, SROWS], F32, tag="psT",
                                     name=f"psT_{it}")
                    for k in range(KT):
                        wsb = w_sbA if k < 8 else w_sbB
                        ko = k if k < 8 else k - 8
                        nc.tensor.matmul(
                            psT[:],
                            wsb[:, ko * KCLS:(ko + 1) * KCLS],
                            xst_sb[:, k * SROWS:(k + 1) * SROWS],
                            start=(k == 0), stop=False)
                    nc.tensor.matmul(psT[:], wb_bf[:], ones_r[:],
                                     start=False, stop=True)

                    # ---- hinge: gl = oh*ssum - stepb = -NK*G ----
                    for m in range(RT):
                        nc.vector.tensor_copy(
                            sT_sb[:, m * 128:(m + 1) * 128],
                            psT[:, m * 128:(m + 1) * 128])
                        psm = ps_m.tile([128, 128], BF, tag="psm",
                                        name=f"psm_{it}_{m}")
                        nc.tensor.transpose(
                            psm[:], sT_sb[:, m * 128:(m + 1) * 128],
                            id_bf[:])
                        ohm = oh_sb[:, m * KCLS:(m + 1) * KCLS]
                        junk = scr.tile([128, KCLS], BF, tag="junk",
                                        name=f"junk_{it}_{m}")
                        corr = sm.tile([128, 1], F32, tag="corr",
                                       name=f"corr_{it}_{m}")
                        ssum = sm.tile([128, 1], F32, tag="ssum",
                                       name=f"ssum_{it}_{m}")
                        stepb = scr.tile([128, KCLS], BF, tag="stepb",
                                         name=f"stepb_{it}_{m}")
                        nc.vector.scalar_tensor_tensor(
                            out=junk[:], in0=psm[:], scalar=1.0,
                            in1=ohm, op0=ALU.mult, op1=ALU.mult,
                            accum_out=corr[:])
                        nc.vector.tensor_scalar(
                            out=stepb[:], in0=psm[:],
                            scalar1=corr[:], scalar2=-1.0,
                            op0=ALU.subtract, op1=ALU.is_gt)
                        nc.vector.tensor_reduce(
                            out=ssum[:], in_=stepb[:],
                            axis=mybir.AxisListType.X, op=ALU.add)
                        nc.vector.scalar_tensor_tensor(
                            out=gl_sb[:, m * KCLS:(m + 1) * KCLS],
                            in0=ohm, scalar=ssum[:], in1=stepb[:],
                            op0=ALU.mult, op1=ALU.subtract)

                    # ---- -NK*gradT chunks + bias grad; pack halves ----
                    gin0 = dram.tile([128, HW_], BF, tag=f"gi0_{it}",
                                     name=f"gi0_{it}")
                    gin1 = dram.tile([128, HW_], BF, tag=f"gi1_{it}",
                                     name=f"gi1_{it}")
                    rs0 = dram.tile([128, 128], BF, tag=f"rs0_{it}",
                                    name=f"rs0_{it}")
                    rs1 = dram.tile([128, 128], BF, tag=f"rs1_{it}",
                                    name=f"rs1_{it}")
                    agw0 = dram.tile([128, 128], BF, tag=f"aw0_{it}",
                                     name=f"aw0_{it}")
                    agw1 = dram.tile([128, 128], BF, tag=f"aw1_{it}",
                                     name=f"aw1_{it}")
                    wout0 = dram.tile([128, HW_], BF, addr_space="Shared",
                                      tag=f"wo0_{it}", name=f"wo0_{it}")
                    wout1 = dram.tile([128, HW_], BF, addr_space="Shared",
                                      tag=f"wo1_{it}", name=f"wo1_{it}")
                    gbin = dram.tile([128, 8], BF, tag=f"gb_{it}",
                                     name=f"gb_{it}")
                    gbout = dram.tile([128, 8], BF, addr_space="Shared",
                                      tag=f"gbo_{it}", name=f"gbo_{it}")
                    psgb = ps_b.tile([128, 1], F32, tag="psgb",
                                     name=f"psgb_{it}")
                    for c in range(4):
                        psg = ps_g.tile([128, 512], F32, tag="psg",
                                        name=f"psg_{it}_{c}")
                        for k in range(RT):
                            nc.tensor.matmul(
                                psg[:],
                                gl_sb[:, k * KCLS:(k + 1) * KCLS],
                                xloc_sb[:, k * D + c * 512:
                                        k * D + (c + 1) * 512],
                                start=(k == 0), stop=(k == RT - 1))
                        if c == 1:
                            for k in range(RT):
                                nc.tensor.matmul(
                                    psgb[:],
                                    gl_sb[:, k * KCLS:(k + 1) * KCLS],
                                    ones_c[:],
                                    start=(k == 0), stop=(k == RT - 1))
                        gpk = gpk0 if c < 2 else gpk1
                        nc.scalar.copy(
                            gpk[:, (c % 2) * 512:(c % 2) * 512 + 512],
                            psg[:])
                        if c == 1:
                            nc.scalar.copy(gbk[:, 0:1], psgb[:])
                            nc.sync.dma_start(gin0[:], gpk0[:])
                            nc.gpsimd.collective_compute(
                                "ReduceScatter", ALU.add,
                                replica_groups=GROUP,
                                ins=[gin0[:]], outs=[rs0[:]])
                        if c == 3:
                            nc.sync.dma_start(gin1[:], gpk1[:])

                    # ---- half A: update 16-class W^T slice, AG, xbar ----
                    nc.sync.dma_start(gsA[:], rs0[:])
                    nc.vector.tensor_scalar_mul(wts0[:], wts0[:], DECAY)
                    nc.vector.scalar_tensor_tensor(
                        out=wts0[:], in0=gsA[:], scalar=LRNK, in1=wts0[:],
                        op0=ALU.mult, op1=ALU.add)
                    nc.vector.tensor_copy(wc0[:], wts0[:])
                    nc.sync.dma_start(agw0[:], wc0[:])
                    nc.gpsimd.collective_compute(
                        "AllGather", ALU.bypass, replica_groups=GROUP,
                        ins=[agw0[:]], outs=[wout0[:]])
                    nc.sync.dma_start_transpose(
                        w_sbA[:].rearrange("p (k c) -> p k c", c=128),
                        wout0[:])

                    # ---- half B (+bias): RS, update, AG, xbar ----
                    nc.gpsimd.collective_compute(
                        "ReduceScatter", ALU.add, replica_groups=GROUP,
                        ins=[gin1[:]], outs=[rs1[:]])
                    nc.sync.dma_start(gsB[:], rs1[:])
                    nc.vector.tensor_scalar_mul(wts1[:], wts1[:], DECAY)
                    nc.vector.scalar_tensor_tensor(
                        out=wts1[:], in0=gsB[:], scalar=LRNK, in1=wts1[:],
                        op0=ALU.mult, op1=ALU.add)
                    nc.vector.tensor_copy(wc1[:], wts1[:])
                    nc.sync.dma_start(agw1[:], wc1[:])
                    nc.gpsimd.collective_compute(
                        "AllGather", ALU.bypass, replica_groups=GROUP,
                        ins=[agw1[:]], outs=[wout1[:]])
                    nc.sync.dma_start_transpose(
                        w_sbB[:].rearrange("p (k c) -> p k c", c=128),
                        wout1[:])

                    # ---- bias: tiny AllReduce of gradb, row update ----
                    nc.sync.dma_start(gbin[:], gbk[:])
                    nc.gpsimd.collective_compute(
                        "AllReduce", ALU.add, replica_groups=GROUP,
                        ins=[gbin[:]], outs=[gbout[:]])
                    nc.sync.dma_start(
                        gbfl[:], gbout[:].rearrange("(a p) f -> a (p f)",
                                                    a=1))
                    nc.vector.tensor_copy(
                        gbst[:],
                        gbfl[:].rearrange("a (c j) -> a c j", j=8)[:, :, 0])
                    nc.vector.scalar_tensor_tensor(
                        out=wb_f32[:], in0=gbst[:], scalar=LRNK,
                        in1=wb_f32[:], op0=ALU.mult, op1=ALU.add)
                    nc.vector.tensor_copy(wb_bf[:], wb_f32[:])

                    # ---- Q^T prefetch: one k-slice per iteration ----
                    for k in range(KT):
                        if k % (ITERS - 1) == it - 1:
                            nc.scalar.dma_start(
                                qt_sb[:, k * QROWS:(k + 1) * QROWS],
                                qtt[:, k * QROWS:(k + 1) * QROWS])

            # ---- query phase: out^T = W^T Q^T + b ----
            with (
                tc.tile_pool(name="qout", bufs=2) as qout,
                tc.tile_pool(name="ps_q", bufs=1, space="PSUM") as ps_q,
            ):
                NCHUNK = QROWS // 512
                pqs = [ps_q.tile([128, 512], F32, tag=f"pq{ch}",
                                 name=f"pq_{ch}") for ch in range(NCHUNK)]
                for k in range(KT):
                    wsb = w_sbA if k < 8 else w_sbB
                    ko = k if k < 8 else k - 8
                    for ch in range(NCHUNK):
                        nc.tensor.matmul(
                            pqs[ch][:],
                            wsb[:, ko * KCLS:(ko + 1) * KCLS],
                            qt_sb[:, k * QROWS + ch * 512:
                                  k * QROWS + (ch + 1) * 512],
                            start=(k == 0), stop=False)
                for ch in range(NCHUNK):
                    nc.tensor.matmul(pqs[ch][:], wb_bf[:],
                                     ones_r[:, 0:512],
                                     start=False, stop=True)
                    qo = qout.tile([128, 512], F32, tag="qo",
                                   name=f"qo_{ch}")
                    nc.vector.tensor_copy(qo[:], pqs[ch][:])
                    nc.sync.dma_start(
                        outT[:, ch * 512:(ch + 1) * 512], qo[:])
    nc.compile()
    return nc


def _tile128(a, p=128):
    """[K*p, F] row-major -> [p, K*F] k-tile SBUF layout."""
    k = a.shape[0] // p
    return np.ascontiguousarray(
        a.reshape(k, p, a.shape[1]).transpose(1, 0, 2).reshape(p, -1))


def _fold(a):
    """[16, 1024] -> [128, 128] folded (partition = r*8 + chunk)."""
    return np.ascontiguousarray(a.reshape(CS, 8, 128).reshape(128, 128))


def _prep_inputs(support_embeddings, support_labels, query_embeddings):
    X = np.asarray(support_embeddings, dtype=np.float32)
    labels = np.asarray(support_labels).astype(np.int64)
    Q = np.asarray(query_embeddings, dtype=np.float32)

    oh_full = (labels[:, None] == np.arange(KCLS)[None, :]).astype(np.float32)
    # host iteration 0: W=0 -> G0 = 1 - KCLS*oh (unscaled); W1 = -(LR/NK) Xb^T G0
    g0 = 1.0 - np.float32(KCLS) * oh_full
    w1 = (-LR / NK) * (X.T.astype(np.float32) @ g0)        # [2048, 128]
    w1bias = (-LR / NK) * g0.sum(axis=0, keepdims=True)    # [1, 128]

    w1bf = _tile128(w1.astype(BF16))                       # [128, 2048]
    w1a = np.ascontiguousarray(w1bf[:, :HW_])
    w1b_ = np.ascontiguousarray(w1bf[:, HW_:])
    w1brf = np.ascontiguousarray(w1bias.astype(np.float32))
    w1brb = np.ascontiguousarray(w1bias.astype(BF16))
    w1T = np.ascontiguousarray(w1.T)                       # [128, 2048] f32

    in_maps = []
    for c in range(NCORES):
        rs, re = c * SROWS, (c + 1) * SROWS
        qs, qe = c * QROWS, (c + 1) * QROWS
        Xc = X[rs:re]
        sl = w1T[c * CS:(c + 1) * CS]                      # [16, 2048]
        in_maps.append({
            "xst": _tile128(np.ascontiguousarray(Xc.T).astype(BF16)),
            "xloc": _tile128(Xc.astype(BF16)),
            "oht": _tile128(oh_full[rs:re].astype(BF16)),
            "qtt": _tile128(np.ascontiguousarray(Q[qs:qe].T).astype(BF16)),
            "w1a": w1a, "w1b_": w1b_,
            "w1t0": _fold(sl[:, :HW_].astype(np.float32)),
            "w1t1": _fold(sl[:, HW_:].astype(np.float32)),
            "w1brf": w1brf, "w1brb": w1brb,
        })
    return in_maps


_NC_CACHE = None


def kernel(support_embeddings, support_labels, query_embeddings,
           n_classes=KCLS, **_):
    global _NC_CACHE
    if _NC_CACHE is None:
        _NC_CACHE = build()
    nc = _NC_CACHE
    in_maps = _prep_inputs(support_embeddings, support_labels,
                           query_embeddings)
    trace = bool(os.environ.get("KERNEL_TRACE"))
    res = run_bass_kernel_spmd(nc, in_maps, core_ids=list(range(NCORES)),
                               trace=trace)
    if trace and res.exec_time_ns is not None:
        print(f"HW exec time: {res.exec_time_ns} ns")
    out = np.concatenate(
        [res.results[c]["outT"].T for c in range(NCORES)], axis=0)
    return np.ascontiguousarray(out.astype(np.float32))


# revision 9
# speedup vs baseline: 1.0892x; 1.0892x over previous
"""Differentiable SVM on 8 trn2 cores — v4: one RS + one AG per iteration.

Per iteration: local partial grad (classes-major [128, 2048] + bias col in a
padded 128-block -> blob [128, 2176]); ONE ReduceScatter (flat split = 16
class-rows per core) sums partials; the core updates its [16, 2049] W^T
slice (f32 master) and casts; ONE AllGather rebuilds the bf16 W^T [128,
2176]; one xbar dma_start_transpose lands it as embed-major k-tiles + bias
row. Collectives have a ~4-7us floor on this transport, so count rules.
"""
import os

import numpy as np
import ml_dtypes

import concourse.bass as bass
import concourse.bacc as bacc
import concourse.masks as masks
import concourse.mybir as mybir
import concourse.tile as tile
from concourse.bass_utils import run_bass_kernel_spmd

BF16 = ml_dtypes.bfloat16
F32 = mybir.dt.float32
BF = mybir.dt.bfloat16
ALU = mybir.AluOpType

NCORES = 8
N_SUP = 4096
D = 2048
KCLS = 128
N_Q = 16384
SROWS = N_SUP // NCORES
QROWS = N_Q // NCORES
ITERS = 15
LR = np.float32(0.01)
CREG = np.float32(1.0)
NK = np.float32(N_SUP * KCLS)
DECAY = float(np.float32(1.0) - LR * CREG)
LRNK = float(LR / NK)
KT = D // 128               # 16 embed k-tiles
RT = SROWS // 128           # 4 support-row k-tiles
WB_ = D + KCLS              # 2176 blob width (embed + bias block)
CS = KCLS // NCORES         # 16-class slice per core
GROUP = [list(range(NCORES))]


def build():
    nc = bacc.Bacc("TRN2", target_bir_lowering=False, debug=False,
                   num_devices=NCORES)

    xst = nc.dram_tensor("xst", [128, KT * SROWS], BF, kind="ExternalInput")
    xloc = nc.dram_tensor("xloc", [128, RT * D], BF, kind="ExternalInput")
    oht = nc.dram_tensor("oht", [128, RT * KCLS], BF, kind="ExternalInput")
    qtt = nc.dram_tensor("qtt", [128, KT * QROWS], BF, kind="ExternalInput")
    w1sb = nc.dram_tensor("w1sb", [128, WB_], BF, kind="ExternalInput")
    w1ts = nc.dram_tensor("w1ts", [CS, D + 1], F32, kind="ExternalInput")
    outT = nc.dram_tensor("outT", [KCLS, QROWS], F32, kind="ExternalOutput")

    with tile.TileContext(nc) as tc:
        with (
            tc.tile_pool(name="static", bufs=1) as st,
            tc.tile_pool(name="dram", bufs=1, space="DRAM") as dram,
            tc.tile_pool(name="small", bufs=8) as sm,
            tc.tile_pool(name="scratch", bufs=4) as scr,
        ):
            xst_sb = st.tile([128, KT * SROWS], BF)
            xloc_sb = st.tile([128, RT * D], BF)
            oh_sb = st.tile([128, RT * KCLS], BF)
            qt_sb = st.tile([128, KT * QROWS], BF)
            w_sb = st.tile([128, WB_], BF)      # W k-tiles + bias block
            wts = st.tile([CS, D + 1], F32)     # W^T class-slice master
            wcs = st.tile([CS, WB_], BF)        # cast staging (AG payload)
            gsF = st.tile([CS, WB_], BF)        # RS result slice
            gl_sb = st.tile([128, RT * KCLS], BF)
            sT_sb = st.tile([128, SROWS], BF)
            gpk = st.tile([128, WB_], BF)       # RS pack blob
            ones_c = st.tile([128, 1], BF)
            ones_r = st.tile([1, SROWS], BF)
            id_bf = st.tile([128, 128], BF)

            nc.vector.memset(gpk[:, D + 1:WB_], 0.0)
            nc.vector.memset(wcs[:, D + 1:WB_], 0.0)
            nc.vector.memset(ones_c[:], 1.0)
            nc.vector.memset(ones_r[:], 1.0)
            masks.make_identity(nc, id_bf[:])

            nc.sync.dma_start(w_sb[:], w1sb[:])
            for lo, hi in ((0, 4), (4, 8), (8, 12), (12, 16)):
                nc.sync.dma_start(xst_sb[:, lo * SROWS:hi * SROWS],
                                  xst[:, lo * SROWS:hi * SROWS])
            nc.sync.dma_start(oh_sb[:], oht[:])
            for lo, hi in ((0, 2), (2, 4)):
                nc.scalar.dma_start(xloc_sb[:, lo * D:hi * D],
                                    xloc[:, lo * D:hi * D])
            nc.scalar.dma_start(wts[:], w1ts[:])

            with (
                tc.tile_pool(name="ps_sc", bufs=1, space="PSUM") as ps_sc,
                tc.tile_pool(name="ps_m", bufs=2, space="PSUM") as ps_m,
                tc.tile_pool(name="ps_g", bufs=2, space="PSUM") as ps_g,
                tc.tile_pool(name="ps_b", bufs=1, space="PSUM") as ps_b,
            ):
                for it in range(1, ITERS):
                    # scores^T = W^T X_c^T + b
                    psT = ps_sc.tile([128, SROWS], F32, tag="psT",
                                     name=f"psT_{it}")
                    for k in range(KT):
                        nc.tensor.matmul(
                            psT[:],
                            w_sb[:, k * KCLS:(k + 1) * KCLS],
                            xst_sb[:, k * SROWS:(k + 1) * SROWS],
                            start=(k == 0), stop=False)
                    nc.tensor.matmul(psT[:], w_sb[0:1, D:WB_], ones_r[:],
                                     start=False, stop=True)

                    # hinge -> gl = -NK*G
                    for m in range(RT):
                        nc.vector.tensor_copy(
                            sT_sb[:, m * 128:(m + 1) * 128],
                            psT[:, m * 128:(m + 1) * 128])
                        psm = ps_m.tile([128, 128], BF, tag="psm",
                                        name=f"psm_{it}_{m}")
                        nc.tensor.transpose(
                            psm[:], sT_sb[:, m * 128:(m + 1) * 128],
                            id_bf[:])
                        ohm = oh_sb[:, m * KCLS:(m + 1) * KCLS]
                        junk = scr.tile([128, KCLS], BF, tag="junk",
                                        name=f"junk_{it}_{m}")
                        corr = sm.tile([128, 1], F32, tag="corr",
                                       name=f"corr_{it}_{m}")
                        ssum = sm.tile([128, 1], F32, tag="ssum",
                                       name=f"ssum_{it}_{m}")
                        stepb = scr.tile([128, KCLS], BF, tag="stepb",
                                         name=f"stepb_{it}_{m}")
                        nc.vector.scalar_tensor_tensor(
                            out=junk[:], in0=psm[:], scalar=1.0,
                            in1=ohm, op0=ALU.mult, op1=ALU.mult,
                            accum_out=corr[:])
                        nc.vector.tensor_scalar(
                            out=stepb[:], in0=psm[:],
                            scalar1=corr[:], scalar2=-1.0,
                            op0=ALU.subtract, op1=ALU.is_gt)
                        nc.vector.tensor_reduce(
                            out=ssum[:], in_=stepb[:],
                            axis=mybir.AxisListType.X, op=ALU.add)
                        nc.vector.scalar_tensor_tensor(
                            out=gl_sb[:, m * KCLS:(m + 1) * KCLS],
                            in0=ohm, scalar=ssum[:], in1=stepb[:],
                            op0=ALU.mult, op1=ALU.subtract)

                    # -NK*gradT chunks + bias col; one blob
                    gin = dram.tile([128, WB_], BF, tag=f"gi_{it}",
                                    name=f"gi_{it}")
                    rsf = dram.tile([CS, WB_], BF, tag=f"rs_{it}",
                                    name=f"rs_{it}")
                    agw = dram.tile([CS, WB_], BF, tag=f"aw_{it}",
                                    name=f"aw_{it}")
                    wout = dram.tile([128, WB_], BF, addr_space="Shared",
                                     tag=f"wo_{it}", name=f"wo_{it}")
                    psgb = ps_b.tile([128, 1], F32, tag="psgb",
                                     name=f"psgb_{it}")
                    for c in range(4):
                        psg = ps_g.tile([128, 512], F32, tag="psg",
                                        name=f"psg_{it}_{c}")
                        for k in range(RT):
                            nc.tensor.matmul(
                                psg[:],
                                gl_sb[:, k * KCLS:(k + 1) * KCLS],
                                xloc_sb[:, k * D + c * 512:
                                        k * D + (c + 1) * 512],
                                start=(k == 0), stop=(k == RT - 1))
                        if c == 1:
                            for k in range(RT):
                                nc.tensor.matmul(
                                    psgb[:],
                                    gl_sb[:, k * KCLS:(k + 1) * KCLS],
                                    ones_c[:],
                                    start=(k == 0), stop=(k == RT - 1))
                        nc.scalar.copy(gpk[:, c * 512:(c + 1) * 512],
                                       psg[:])
                        if c == 1:
                            nc.scalar.copy(gpk[:, D:D + 1], psgb[:])
                    nc.sync.dma_start(gin[:], gpk[:])
                    nc.gpsimd.collective_compute(
                        "ReduceScatter", ALU.add, replica_groups=GROUP,
                        ins=[gin[:]], outs=[rsf[:]])

                    # slice update (classes-rows [16, 2049])
                    nc.sync.dma_start(gsF[:], rsf[:])
                    nc.vector.tensor_scalar_mul(
                        wts[:, 0:D], wts[:, 0:D], DECAY)
                    nc.vector.scalar_tensor_tensor(
                        out=wts[:], in0=gsF[:, 0:D + 1], scalar=LRNK,
                        in1=wts[:], op0=ALU.mult, op1=ALU.add)
                    nc.vector.tensor_copy(wcs[:, 0:D + 1], wts[:])
                    nc.sync.dma_start(agw[:], wcs[:])
                    nc.gpsimd.collective_compute(
                        "AllGather", ALU.bypass, replica_groups=GROUP,
                        ins=[agw[:]], outs=[wout[:]])
                    nc.sync.dma_start_transpose(
                        w_sb[:].rearrange("p (k c) -> p k c", c=128),
                        wout[:])

                    for k in range(KT):
                        if k % (ITERS - 1) == it - 1:
                            nc.scalar.dma_start(
                                qt_sb[:, k * QROWS:(k + 1) * QROWS],
                                qtt[:, k * QROWS:(k + 1) * QROWS])

            with (
                tc.tile_pool(name="qout", bufs=2) as qout,
                tc.tile_pool(name="ps_q", bufs=1, space="PSUM") as ps_q,
            ):
                NCHUNK = QROWS // 512
                pqs = [ps_q.tile([128, 512], F32, tag=f"pq{ch}",
                                 name=f"pq_{ch}") for ch in range(NCHUNK)]
                for k in range(KT):
                    for ch in range(NCHUNK):
                        nc.tensor.matmul(
                            pqs[ch][:],
                            w_sb[:, k * KCLS:(k + 1) * KCLS],
                            qt_sb[:, k * QROWS + ch * 512:
                                  k * QROWS + (ch + 1) * 512],
                            start=(k == 0), stop=False)
                for ch in range(NCHUNK):
                    nc.tensor.matmul(pqs[ch][:], w_sb[0:1, D:WB_],
                                     ones_r[:, 0:512],
                                     start=False, stop=True)
                    qo = qout.tile([128, 512], F32, tag="qo",
                                   name=f"qo_{ch}")
                    nc.vector.tensor_copy(qo[:], pqs[ch][:])
                    nc.sync.dma_start(
                        outT[:, ch * 512:(ch + 1) * 512], qo[:])
    nc.compile()
    return nc


def _tile128(a, p=128):
    k = a.shape[0] // p
    return np.ascontiguousarray(
        a.reshape(k, p, a.shape[1]).transpose(1, 0, 2).reshape(p, -1))


def _prep_inputs(support_embeddings, support_labels, query_embeddings):
    X = np.asarray(support_embeddings, dtype=np.float32)
    labels = np.asarray(support_labels).astype(np.int64)
    Q = np.asarray(query_embeddings, dtype=np.float32)

    oh_full = (labels[:, None] == np.arange(KCLS)[None, :]).astype(np.float32)
    g0 = 1.0 - np.float32(KCLS) * oh_full
    w1 = (-LR / NK) * (X.T.astype(np.float32) @ g0)        # [2048, 128]
    w1bias = (-LR / NK) * g0.sum(axis=0, keepdims=True)    # [1, 128]

    w1sb = np.empty((128, WB_), BF16)
    w1sb[:, :D] = _tile128(w1.astype(BF16))
    w1sb[:, D:] = np.broadcast_to(w1bias.astype(BF16), (128, KCLS))
    w1sb = np.ascontiguousarray(w1sb)
    w1Tf = np.concatenate([w1, w1bias], axis=0).T          # [128, 2049] f32
    w1Tf = np.ascontiguousarray(w1Tf.astype(np.float32))

    in_maps = []
    for c in range(NCORES):
        rs, re = c * SROWS, (c + 1) * SROWS
        qs, qe = c * QROWS, (c + 1) * QROWS
        Xc = X[rs:re]
        in_maps.append({
            "xst": _tile128(np.ascontiguousarray(Xc.T).astype(BF16)),
            "xloc": _tile128(Xc.astype(BF16)),
            "oht": _tile128(oh_full[rs:re].astype(BF16)),
            "qtt": _tile128(np.ascontiguousarray(Q[qs:qe].T).astype(BF16)),
            "w1sb": w1sb,
            "w1ts": np.ascontiguousarray(w1Tf[c * CS:(c + 1) * CS]),
        })
    return in_maps


_NC_CACHE = None


def kernel(support_embeddings, support_labels, query_embeddings,
           n_classes=KCLS, **_):
    global _NC_CACHE
    if _NC_CACHE is None:
        _NC_CACHE = build()
    nc = _NC_CACHE
    in_maps = _prep_inputs(support_embeddings, support_labels,
                           query_embeddings)
    trace = bool(os.environ.get("KERNEL_TRACE"))
    res = run_bass_kernel_spmd(nc, in_maps, core_ids=list(range(NCORES)),
                               trace=trace)
    if trace and res.exec_time_ns is not None:
        print(f"HW exec time: {res.exec_time_ns} ns")
    out = np.concatenate(
        [res.results[c]["outT"].T for c in range(NCORES)], axis=0)
    return np.ascontiguousarray(out.astype(np.float32))


# revision 10
# speedup vs baseline: 1.3136x; 1.2060x over previous
"""Differentiable SVM (hinge-loss GD + linear predict) on 8 Trainium2 cores.

Strategy:
  - Support rows sharded 512/core (scores + local G), V rows sharded 256/core
    (gradient slice). Per GD iteration three 64KB AllGathers (Mesh algo):
    G in two 256-row halves (pipelined against compute) and V.
  - gradb is folded into the gradV^T matmul via a ones-column appended to
    xcol; bias adds are folded into DVE copies as per-partition scalars
    (b master is [classes, 1]); V^T/b masters stay f32 per-core.
  - Iteration 0 (W=0) uses the closed-form G0 = 1 - n_classes*onehot passed
    as a constant input, skipping the scores matmuls and both G AllGathers.
  - scores computed transposed (matmuls of N=256) then PE-transposed back;
    gradV computed transposed (32 matmuls of N=257, incl. gradb column).
  - Query matmul computes out^T = W^T @ Q^T with Q^T prepared host-side in
    bf16 and prefetched to SBUF during the fit; host transposes the result.
"""
import os

import numpy as np
import ml_dtypes

import concourse.bass as bass
import concourse.bacc as bacc
import concourse.masks as masks
import concourse.mybir as mybir
import concourse.tile as tile
from concourse.bass_utils import run_bass_kernel_spmd

BF16 = ml_dtypes.bfloat16
F32 = mybir.dt.float32
BF = mybir.dt.bfloat16
ALU = mybir.AluOpType

NCORES = 8
N_SUP = 4096        # support rows
D = 2048            # embed dim (no bias)
KCLS = 128          # n_classes
N_Q = 16384         # query rows
SROWS = N_SUP // NCORES      # 512 support rows / core  (4 row tiles)
HROWS = SROWS // 2           # 256-row half-shards for the G AllGathers
VROWS = D // NCORES          # 256 V rows / core        (2 m tiles)
QROWS = N_Q // NCORES        # 2048 query rows / core   (4 chunks of 512)
ITERS = 15
LR = np.float32(0.01)
CREG = np.float32(1.0)
NK = np.float32(N_SUP * KCLS)            # 524288 = 2**19 (exact)
DECAY = float(np.float32(1.0) - LR * CREG)   # 0.99 (f32 rounded)
LRNK = float(LR / NK)                    # 0.01 / 2**19

KT_E = D // 128      # 16 embed k-tiles
KT_R = N_SUP // 128  # 32 support-row k-tiles
RT = SROWS // 128    # 4 local row tiles
MT = VROWS // 128    # 2 V m-tiles per core
XCW = VROWS + 1      # xcol width incl. ones column (gradb fold)
GROUP = [list(range(NCORES))]


def build():
    nc = bacc.Bacc("TRN2", target_bir_lowering=False, debug=False,
                   num_devices=NCORES)

    xst = nc.dram_tensor("xst", [128, KT_E * SROWS], BF, kind="ExternalInput")
    xcol = nc.dram_tensor("xcol", [128, KT_R * XCW], BF, kind="ExternalInput")
    oh = nc.dram_tensor("oh", [128, RT * KCLS], BF, kind="ExternalInput")
    w1w = nc.dram_tensor("w1w", [128, KT_E * KCLS], BF, kind="ExternalInput")
    w1v = nc.dram_tensor("w1v", [128, XCW], F32, kind="ExternalInput")
    qt = nc.dram_tensor("qt", [D, QROWS], BF, kind="ExternalInput")
    outT = nc.dram_tensor("outT", [KCLS, QROWS], F32, kind="ExternalOutput")

    with tile.TileContext(nc) as tc:
        with (
            tc.tile_pool(name="static", bufs=1) as st,
            tc.tile_pool(name="dram", bufs=1, space="DRAM") as dram,
            tc.tile_pool(name="small", bufs=8) as sm,
            tc.tile_pool(name="scratch", bufs=4) as scr_pool,
        ):
            # ---- static SBUF tensors ----
            xst_sb = st.tile([128, KT_E * SROWS], BF)       # X_s^T
            xcol_sb = st.tile([128, KT_R * XCW], BF)        # X cols + ones
            qt_sb = st.tile([128, KT_E * QROWS], BF)        # Q^T (prefetch)
            oh_sb = st.tile([128, RT * KCLS], BF)           # local one-hot
            w_sb = st.tile([128, KT_E * KCLS], BF)          # v_out mirror
            g_sb = st.tile([128, KT_R * KCLS], BF)          # gathered G
            gl_sb = st.tile([128, RT * KCLS], BF)           # local -G
            vTb = st.tile([128, XCW], F32)                  # [V^T | b] master
            vbf_sb = st.tile([128, MT * KCLS], BF)          # V (AG layout)
            id_f32 = st.tile([128, 128], F32)

            nc.scalar.dma_start(vTb[:], w1v[:])
            nc.sync.dma_start(w_sb[:], w1w[:])
            masks.make_identity(nc, id_f32[:])
            bT = vTb[:, VROWS:XCW]          # [128, 1] f32 bias (by class)


            # ---- initial loads (host pre-tiled, contiguous) ----
            for lo, hi in ((0, 8), (8, 16)):
                nc.sync.dma_start(xst_sb[:, lo * SROWS:hi * SROWS],
                                  xst[:, lo * SROWS:hi * SROWS])
            nc.sync.dma_start(oh_sb[:], oh[:])
            for lo, hi in ((0, 8), (8, 16), (16, 24), (24, 32)):
                nc.sync.dma_start(xcol_sb[:, lo * XCW:hi * XCW],
                                  xcol[:, lo * XCW:hi * XCW])

            with (
                tc.tile_pool(name="ps_big", bufs=2, space="PSUM") as ps_big,
                tc.tile_pool(name="ps_s", bufs=4, space="PSUM") as ps_s,
                tc.tile_pool(name="ps_tr", bufs=2, space="PSUM") as ps_tr,
            ):
                # ---- GD iterations ----
                for it in range(1, ITERS):
                    # ridge decay off the critical tail (V master only)
                    nc.vector.tensor_scalar_mul(
                        vTb[:, 0:VROWS], vTb[:, 0:VROWS], DECAY)
                    if True:
                        # scores^T = W^T X_s^T -> [classes, 512] (one group)
                        psT = ps_big.tile([128, SROWS], F32, tag="big",
                                          name=f"psT_{it}")
                        for k in range(KT_E):
                            nc.tensor.matmul(
                                psT[:],
                                w_sb[:, k * KCLS:(k + 1) * KCLS],
                                xst_sb[:, k * SROWS:(k + 1) * SROWS],
                                start=(k == 0), stop=(k == KT_E - 1))
                        # add bias while copying out of PSUM
                        sT = scr_pool.tile([128, SROWS], F32, tag="sT",
                                           name=f"sT_{it}")
                        for sl in range(RT):
                            nc.vector.tensor_scalar(
                                out=sT[:, sl * 128:(sl + 1) * 128],
                                in0=psT[:, sl * 128:(sl + 1) * 128],
                                scalar1=bT, scalar2=None, op0=ALU.add)
                        for h in range(2):
                            for mm in range(2):
                                m = 2 * h + mm
                                ps = ps_s.tile([128, KCLS], F32,
                                               tag="ps_s",
                                               name=f"ps_s_{it}_{m}")
                                nc.tensor.transpose(
                                    ps[:],
                                    sT[:, m * 128:(m + 1) * 128],
                                    id_f32[:])
                                ohm = oh_sb[:, m * KCLS:(m + 1) * KCLS]
                                scrt = scr_pool.tile(
                                    [128, KCLS], F32, tag="scrt",
                                    name=f"scrt_{it}_{m}")
                                corr = sm.tile([128, 1], F32, tag="corr",
                                               name=f"corr_{it}_{m}")
                                ssum = sm.tile([128, 1], F32, tag="ssum",
                                               name=f"ssum_{it}_{m}")
                                stepb = scr_pool.tile(
                                    [128, KCLS], BF, tag="stepb",
                                    name=f"stepb_{it}_{m}")
                                nc.vector.scalar_tensor_tensor(
                                    out=scrt[:], in0=ps[:], scalar=1.0,
                                    in1=ohm, op0=ALU.mult, op1=ALU.mult,
                                    accum_out=corr[:])
                                nc.vector.tensor_scalar(
                                    out=stepb[:], in0=ps[:],
                                    scalar1=corr[:], scalar2=-1.0,
                                    op0=ALU.subtract, op1=ALU.is_gt)
                                nc.vector.tensor_reduce(
                                    out=ssum[:], in_=stepb[:],
                                    axis=mybir.AxisListType.X, op=ALU.add)
                                # gl = onehot*S - step = -G
                                nc.vector.scalar_tensor_tensor(
                                    out=gl_sb[:, m * KCLS:(m + 1) * KCLS],
                                    in0=ohm, scalar=ssum[:], in1=stepb[:],
                                    op0=ALU.mult, op1=ALU.subtract)
                            # pack + AllGather this half (64KB -> Mesh)
                            g_in = dram.tile([HROWS, KCLS], BF,
                                             tag=f"g_in{it}_{h}",
                                             name=f"g_in{it}_{h}")
                            g_out = dram.tile([NCORES * HROWS, KCLS], BF,
                                              addr_space="Shared",
                                              tag=f"g_out{it}_{h}",
                                              name=f"g_out{it}_{h}")
                            nc.sync.dma_start(
                                g_in[:].rearrange("(t p) f -> p t f",
                                                  p=128),
                                gl_sb[:, 2 * h * KCLS:
                                      (2 * h + 2) * KCLS]
                                .rearrange("p (t f) -> p t f", t=2))
                            nc.gpsimd.collective_compute(
                                "AllGather", ALU.bypass,
                                replica_groups=GROUP,
                                ins=[g_in[:]], outs=[g_out[:]])
                            for lo, hi in ((0, 2), (2, 8), (8, 16)):
                                nc.sync.dma_start(
                                    g_sb[:, (16 * h + lo) * KCLS:
                                         (16 * h + hi) * KCLS]
                                    .rearrange("p (t f) -> p t f",
                                               t=hi - lo),
                                    g_out[lo * 128:hi * 128, :]
                                    .rearrange("(t p) f -> p t f", p=128))

                    # gradV^T (+gradb col) = G^T [X | 1] : [classes, 257]
                    pgT = ps_big.tile([128, XCW], F32, tag="big",
                                      name=f"pgT_{it}")
                    for k in range(KT_R):
                        nc.tensor.matmul(
                            pgT[:],
                            g_sb[:, k * KCLS:(k + 1) * KCLS],
                            xcol_sb[:, k * XCW:(k + 1) * XCW],
                            start=(k == 0), stop=(k == KT_R - 1))
                    # masters: V^T decayed above; b gets no decay
                    nc.vector.scalar_tensor_tensor(
                        out=vTb[:], in0=pgT[:], scalar=LRNK,
                        in1=vTb[:], op0=ALU.mult, op1=ALU.add)
                    for m in range(MT):
                        ptr = ps_tr.tile([128, 128], F32, tag="ptr",
                                         name=f"ptr_{it}_{m}")
                        nc.tensor.transpose(
                            ptr[:], vTb[:, m * 128:(m + 1) * 128],
                            id_f32[:])
                        nc.vector.tensor_copy(
                            vbf_sb[:, m * KCLS:(m + 1) * KCLS], ptr[:])

                    # AllGather V (64KB -> Mesh)
                    v_in = dram.tile([VROWS, KCLS], BF,
                                     tag=f"v_in{it}", name=f"v_in{it}")
                    v_out = dram.tile([D, KCLS], BF, addr_space="Shared",
                                      tag=f"v_out{it}", name=f"v_out{it}")
                    nc.sync.dma_start(
                        v_in[:].rearrange("(m p) f -> p m f", p=128),
                        vbf_sb[:].rearrange("p (m f) -> p m f", m=MT))
                    nc.gpsimd.collective_compute(
                        "AllGather", ALU.bypass, replica_groups=GROUP,
                        ins=[v_in[:]], outs=[v_out[:]])
                    for lo, hi in ((0, 2), (2, 8), (8, 16)):
                        nc.sync.dma_start(
                            w_sb[:, lo * KCLS:hi * KCLS]
                            .rearrange("p (k f) -> p k f", k=hi - lo),
                            v_out[lo * 128:hi * 128, :]
                            .rearrange("(k p) f -> p k f", p=128))

                    # spread Q^T prefetch across iterations
                    nload = max(1, ITERS - 1)
                    for k in range(KT_E):
                        if it >= 1 and k % nload == it - 1 or \
                                (ITERS == 1 and it == 0):
                            nc.scalar.dma_start(
                                qt_sb[:, k * QROWS:(k + 1) * QROWS],
                                qt[k * 128:(k + 1) * 128, :])

            # ---- query phase: out^T = W^T Q^T + b ----
            with (
                tc.tile_pool(name="qout", bufs=2) as qout,
                tc.tile_pool(name="ps_q", bufs=1, space="PSUM") as ps_q,
            ):
                NCHUNK = QROWS // 512
                pqs = [ps_q.tile([128, 512], F32, tag=f"pq{ch}",
                                 name=f"pq_{ch}") for ch in range(NCHUNK)]
                # k-major: each W tile loaded once, dense PE stream
                for k in range(KT_E):
                    for ch in range(NCHUNK):
                        nc.tensor.matmul(
                            pqs[ch][:],
                            w_sb[:, k * KCLS:(k + 1) * KCLS],
                            qt_sb[:, k * QROWS + ch * 512:
                                  k * QROWS + (ch + 1) * 512],
                            start=(k == 0), stop=(k == KT_E - 1))
                for ch in range(NCHUNK):
                    qo = qout.tile([128, 512], F32, tag="qo",
                                   name=f"qo_{ch}")
                    nc.vector.tensor_scalar(
                        out=qo[:], in0=pqs[ch][:], scalar1=bT,
                        scalar2=None, op0=ALU.add)
                    nc.sync.dma_start(
                        outT[:, ch * 512:(ch + 1) * 512], qo[:])
    nc.compile()
    return nc


def _row_perm():
    """Support-row permutation matching the half-shard AllGather layout:
    [h=0: rank blocks' first 256 rows][h=1: rank blocks' last 256 rows]."""
    idx = []
    for h in range(2):
        for r in range(NCORES):
            s = SROWS * r + HROWS * h
            idx.append(np.arange(s, s + HROWS))
    return np.concatenate(idx)


def _tile128(a, p=128):
    """[K*p, F] row-major -> [p, K*F] k-tile SBUF layout."""
    k = a.shape[0] // p
    return np.ascontiguousarray(
        a.reshape(k, p, a.shape[1]).transpose(1, 0, 2).reshape(p, -1))


def _prep_inputs(support_embeddings, support_labels, query_embeddings):
    X = np.asarray(support_embeddings, dtype=np.float32)
    labels = np.asarray(support_labels).astype(np.int64)
    Q = np.asarray(query_embeddings, dtype=np.float32)

    oh_full = (labels[:, None] == np.arange(KCLS)[None, :])
    perm = _row_perm()
    Xp = X[perm]

    # host iteration 0: W=0 -> G0 = 1-128*oh; W1 = -(LR/NK) Xb^T G0
    g0f = 1.0 - np.float32(KCLS) * oh_full.astype(np.float32)
    w1 = (-LR / NK) * (X.T.astype(np.float32) @ g0f)       # [2048, 128]
    w1bias = (-LR / NK) * g0f.sum(axis=0, keepdims=True)   # [1, 128]
    w1w = _tile128(w1.astype(BF16))                        # [128, 16*128]
    w1T = np.ascontiguousarray(w1.T.astype(np.float32))    # [128, 2048]

    in_maps = []
    for c in range(NCORES):
        rs, re = c * SROWS, (c + 1) * SROWS
        vs, ve = c * VROWS, (c + 1) * VROWS
        qs, qe = c * QROWS, (c + 1) * QROWS
        xc = np.empty((N_SUP, XCW), np.float32)
        xc[:, :VROWS] = Xp[:, vs:ve]
        xc[:, VROWS] = 1.0
        w1v = np.empty((128, XCW), np.float32)
        w1v[:, :VROWS] = w1T[:, vs:ve]
        w1v[:, VROWS] = w1bias[0]
        in_maps.append({
            "xst": _tile128(np.ascontiguousarray(X[rs:re, :].T)
                            .astype(BF16)),
            "xcol": _tile128(xc.astype(BF16)),
            "oh": _tile128(oh_full[rs:re].astype(BF16)),
            "w1w": w1w,
            "w1v": np.ascontiguousarray(w1v),
            "qt": np.ascontiguousarray(Q[qs:qe, :].T).astype(BF16),
        })
    return in_maps


_NC_CACHE = None


def kernel(support_embeddings, support_labels, query_embeddings,
           n_classes=KCLS, **_):
    global _NC_CACHE
    if _NC_CACHE is None:
        _NC_CACHE = build()
    nc = _NC_CACHE
    in_maps = _prep_inputs(support_embeddings, support_labels,
                           query_embeddings)
    trace = bool(os.environ.get("KERNEL_TRACE"))
    res = run_bass_kernel_spmd(nc, in_maps, core_ids=list(range(NCORES)),
                               trace=trace)
    if trace and res.exec_time_ns is not None:
        print(f"HW exec time: {res.exec_time_ns} ns")
    out = np.concatenate(
        [res.results[c]["outT"].T for c in range(NCORES)], axis=0)
    return np.ascontiguousarray(out.astype(np.float32))

